# revision 3
# baseline (speedup 1.0000x reference)
"""Embedding lookup (gather) on 8 TRN2 NeuronCores via SWDGE dma_gather.

Strategy: row-shard the table into 32 blocks of 31250 rows (int16-addressable
by dma_gather), 4 blocks per core. The host bucket-sorts the 500K indices by
block, converts the table to bf16 (rel err <= 2^-9, far under the 2e-2 gate),
and each core runs ONE InstDMAGatherAnt per block (~16K descriptors streamed
through the ring) instead of the baseline's 489 serialized indirect DMAs/core
(each paying ~1.7us SWDGE fixed cost for 64KB). The gathered tiles are written
back contiguously via HWDGE; the host de-interleaves, un-permutes, and upcasts
to fp32.

Per core: ~16MB random 256B reads + ~16.4MB contiguous writes.
"""
import sys
import numpy as np
import ml_dtypes

sys.path.insert(0, "/opt/trn_rl_repo")

import concourse.bacc as bacc
import concourse.bass as bass
import concourse.mybir as mybir
import concourse.tile as tile
from concourse import bass_utils, library_config

N_EMB = 1_000_000
D = 128
N_IDX = 500_000
N_CORES = 8
N_BLOCKS = 32
BLOCK = N_EMB // N_BLOCKS          # 31250 rows, < 32768 (int16-addressable)
BPC = N_BLOCKS // N_CORES          # 4 blocks per core
ROWS_PER_CORE = N_EMB // N_CORES   # 125000

_cached = {}


def _build(cap):
    """cap = per-(core,block) index capacity, multiple of 128."""
    if cap in _cached:
        return _cached[cap]
    V = cap // 128                 # gather output column-slots per partition
    S = cap // 16                  # int16 idx columns per block (16-lane wrap)

    nc = bacc.Bacc(
        "TRN2",
        target_bir_lowering=False,
        debug=False,
        enable_asserts=False,
        num_devices=N_CORES,
    )
    idx_dram = nc.dram_tensor(
        "idx", [128, BPC * S], mybir.dt.int16, kind="ExternalInput"
    ).ap()
    w_dram = nc.dram_tensor(
        "w", [ROWS_PER_CORE, D], mybir.dt.bfloat16, kind="ExternalInput"
    ).ap()
    out = nc.dram_tensor(
        "out", [128, BPC, V, D], mybir.dt.bfloat16, kind="ExternalOutput"
    ).ap()

    with tile.TileContext(nc) as tc:
        with (
            tc.tile_pool(name="idxp", bufs=1) as idxp,
            tc.tile_pool(name="pool", bufs=2) as pool,
        ):
            nc.gpsimd.load_library(library_config.mlp)
            idx_all = idxp.tile([128, BPC * S], mybir.dt.int16)
            nc.sync.dma_start(out=idx_all[:, :], in_=idx_dram[:, :])
            for t in range(BPC):
                g = pool.tile([128, V, D], mybir.dt.bfloat16, tag="g")
                nc.gpsimd.dma_gather(
                    g[:, :, :],
                    w_dram[t * BLOCK:(t + 1) * BLOCK, :],
                    idx_all[:, t * S:(t + 1) * S],
                    cap,
                    cap,
                    D,
                    # >63 descriptors per SDMA engine overflows the single
                    # packet and wedges the device (NRT_EXEC_UNIT_UNRECOVERABLE)
                    single_packet=False,
                )
                nc.sync.dma_start(out=out[:, t, :, :], in_=g[:, :, :])

    nc.compile()
    _cached[cap] = nc
    return nc


def _prepare(input, weight):
    v = np.asarray(input).astype(np.int64)
    w16 = np.asarray(weight, dtype=np.float32).astype(ml_dtypes.bfloat16)

    key = (v // BLOCK).astype(np.int32)
    order = np.argsort(key, kind="stable")
    counts = np.bincount(key, minlength=N_BLOCKS)
    cap = max(int(-(-counts.max() // 128) * 128), 128)
    S = cap // 16

    local = (v[order] - key[order].astype(np.int64) * BLOCK).astype(np.int16)
    # Pad each block's bucket to cap with index 0 (valid row; discarded later).
    idx_pack = np.zeros((N_BLOCKS, cap), dtype=np.int16)
    off = 0
    for g in range(N_BLOCKS):
        n = int(counts[g])
        idx_pack[g, :n] = local[off:off + n]
        off += n
    # Stream slot j lives at [j % 16, j // 16]; replicate across the 8
    # partition groups for the 8 Q7 cores.
    wrapped = idx_pack.reshape(N_BLOCKS, S, 16).transpose(0, 2, 1)  # [g,16,S]
    rep = np.tile(wrapped, (1, 8, 1))                               # [g,128,S]
    idx_cores = (
        rep.reshape(N_CORES, BPC, 128, S)
        .transpose(0, 2, 1, 3)
        .reshape(N_CORES, 128, BPC * S)
    )
    w_cores = w16.reshape(N_CORES, ROWS_PER_CORE, D)
    return idx_cores, w_cores, order, counts, cap


def _unpack(results, order, counts, cap):
    parts = []
    for c in range(N_CORES):
        O = results[c]["out"]  # [128, BPC, V, D] bf16
        for t in range(BPC):
            g = c * BPC + t
            # stream slot j landed at [j % 128, t, j // 128, :]
            G = O[:, t].transpose(1, 0, 2).reshape(cap, D)
            parts.append(G[: int(counts[g])])
    sorted_rows = np.concatenate(parts, axis=0).astype(np.float32)
    out = np.empty((N_IDX, D), dtype=np.float32)
    out[order] = sorted_rows
    return out


def kernel(input, weight, _trace=False, _tmpdir=None):
    idx_cores, w_cores, order, counts, cap = _prepare(input, weight)
    nc = _build(cap)

    in_maps = [
        {
            "idx": np.ascontiguousarray(idx_cores[c]),
            "w": np.ascontiguousarray(w_cores[c]),
        }
        for c in range(N_CORES)
    ]
    res = bass_utils.run_bass_kernel_spmd(
        nc,
        in_maps,
        core_ids=list(range(N_CORES)),
        trace=_trace,
        tmpdir=_tmpdir,
    )
    out = _unpack(res.results, order, counts, cap)
    if _trace:
        return out, res
    return out


# revision 4
# speedup vs baseline: 2.7298x; 2.7298x over previous
"""Embedding lookup (gather) on 8 TRN2 NeuronCores.

Strategy: replicate the 1M x 128 table (bf16) to every core's HBM and shard
the 500K indices 8 ways. Each core gathers its 62.5K rows with 489 indirect
(SWDGE) DMAs of 128 rows each, round-robined across 2 SWDGE queues so two Q7
core-pairs generate descriptors in parallel. Rows move as bf16 (256B) halving
both gather and write-back bytes vs fp32; host upcasts to fp32 (rel err
<= 2^-9, far below the 2e-2 gate).

Index layout per core: the 62592 (padded) indices are reshaped row-major to
[128, 489]; one indirect_dma_start per column gathers 128 rows (one per
partition); write-back to DRAM is one contiguous DMA per 123-column chunk.
"""
import sys
import numpy as np
import ml_dtypes

sys.path.insert(0, "/opt/trn_rl_repo")

import concourse.bacc as bacc
import concourse.bass as bass
import concourse.mybir as mybir
import concourse.tile as tile
from concourse import bass_utils

N_EMB = 1_000_000
D = 128
N_IDX = 500_000
N_CORES = 8
N_QUEUES = 2

P = 128                      # SBUF partitions
COLS = 489                   # index columns per partition: 128*489 = 62592 rows/core
ROWS_PER_CORE = P * COLS     # 62592
PAD_TOTAL = N_CORES * ROWS_PER_CORE  # 500736

# Column chunks per write-back DMA. C*D*2 bytes/partition of SBUF per buffer.
CHUNKS = []
_c0 = 0
for _C in (123, 122, 122, 122):
    CHUNKS.append((_c0, _C))
    _c0 += _C
assert _c0 == COLS

_cached = None


def _build():
    global _cached
    if _cached is not None:
        return _cached

    nc = bacc.Bacc(
        "TRN2",
        target_bir_lowering=False,
        debug=False,
        enable_asserts=False,
        num_devices=N_CORES,
        num_swdge_queues=N_QUEUES,
    )
    idx_dram = nc.dram_tensor(
        "idx", [P, COLS], mybir.dt.int32, kind="ExternalInput"
    ).ap()
    weight = nc.dram_tensor(
        "weight", [N_EMB, D], mybir.dt.bfloat16, kind="ExternalInput"
    ).ap()
    out = nc.dram_tensor(
        "out", [P, COLS * D], mybir.dt.bfloat16, kind="ExternalOutput"
    ).ap()

    with tile.TileContext(nc) as tc:
        with (
            tc.tile_pool(name="idxp", bufs=1) as idxp,
            tc.tile_pool(name="pool", bufs=2) as pool,
        ):
            idx_all = idxp.tile([P, COLS], mybir.dt.int32)
            nc.sync.dma_start(out=idx_all[:, :], in_=idx_dram[:, :])
            for c0, C in CHUNKS:
                g = pool.tile([P, C * D], mybir.dt.bfloat16, tag="g")
                # One indirect DMA per index column: the HW SWDGE ucode uses
                # ONE index per partition per instruction. Round-robin the
                # instructions over the SWDGE queues (queue q is served by Q7
                # core pair 2q/2q+1, so descriptor generation parallelizes).
                for c in range(C):
                    inst = nc.gpsimd.indirect_dma_start(
                        out=g[:, c * D:(c + 1) * D],
                        out_offset=None,
                        in_=weight[:],
                        in_offset=bass.IndirectOffsetOnAxis(
                            ap=idx_all[:, c0 + c:c0 + c + 1], axis=0
                        ),
                    )
                    q = (c0 + c) % N_QUEUES
                    if q:
                        inst.ins.queue = f"qPoolDynamic{q}"
                nc.sync.dma_start(out=out[:, c0 * D:(c0 + C) * D], in_=g[:])

    nc.compile()
    _cached = nc
    return nc


def _prepare(input, weight):
    idx = np.asarray(input).astype(np.int32)
    w16 = np.asarray(weight, dtype=np.float32).astype(ml_dtypes.bfloat16)
    idx_pad = np.zeros(PAD_TOTAL, dtype=np.int32)
    idx_pad[:N_IDX] = idx
    idx_cores = idx_pad.reshape(N_CORES, P, COLS)
    return idx_cores, w16


def kernel(input, weight, _trace=False, _tmpdir=None):
    nc = _build()
    idx_cores, w16 = _prepare(input, weight)

    in_maps = [{"idx": idx_cores[c], "weight": w16} for c in range(N_CORES)]

    res = bass_utils.run_bass_kernel_spmd(
        nc,
        in_maps,
        core_ids=list(range(N_CORES)),
        trace=_trace,
        tmpdir=_tmpdir,
    )

    out = np.concatenate(
        [
            np.asarray(res.results[c]["out"]).reshape(ROWS_PER_CORE, D)
            for c in range(N_CORES)
        ],
        axis=0,
    )[:N_IDX].astype(np.float32)
    if _trace:
        return out, res
    return out
